# revision 2
# baseline (speedup 1.0000x reference)
"""DGF layer kernel v2 for Trainium2 (Bass/Tile), one sample per core.

Reference (per sample, N=1024, D=256, H heads):
    sq[n]  = sum_d x[n,d]^2
    dist2  = sq[n] + sq[m] - 2*x@x.T          (clamped >= 0)
    adj    = mean_h exp(-dist2 / (2*exp(ls_h)^2 + 1e-6))
    out    = LN(elu(adj @ x @ W.T + b) + x) * gamma + beta

v2 strategy (n_alpha==1 fast path):
  - E[m,n] = exp(2a*G[m,n] - a*sq_m - K) computed by ACT from fp8-DoubleRow
    Gram PSUM; diagonal suppressed in PSUM by a -BIG eye matmul. E stored
    fp8e4 and used directly as the stationary operand of the aggregation
    matmul (fp8 DoubleRow, j-block pairs).
  - adjacency identity: sum_m E[m,n]*y[m,d] = e^{a*sq_n+K-lnw} * (adj0@y)[n,d]
    so z = R'[n]*pa[n,d] + y[n,d] with R'[n] = w*exp(K - a*sq_n) recovers
    adj@y including the exact diagonal (+y via PSUM f32, fused in one STT).
  - proj y = x@W.T in fp8 hi/lo (xh@wh + xh@wl + xl@wh), DoubleRow.
  - elu(z)+x = max(z,0) + min(e^z,1) + (x-1); exp on ACT, rest fused STTs.
  - LN stats via fused accum_out (sum v) + tensor_tensor_reduce (sum v^2);
    rstd by 1-iteration quake-seed Newton on DVE/Pool smalls.
  - column-halves pipeline: exp of E columns 0:512 (pass A) -> agg/post of
    row-blocks 0..3 overlaps exp of columns 512:1024 (pass B).
"""

from contextlib import ExitStack

import numpy as np

B, N, D, H = 8, 1024, 256, 8
LN_EPS = 1e-5
P = 128
NT = N // P          # 8 row/col blocks
HALF = N // 2
BIG = 1.0e6

_PROGRAM_CACHE = {}


def _build_fast(a_f, w_f, k_f, need_b, need_gamma, need_beta):
    import concourse.bass as bass
    import concourse.tile as tile
    from concourse import bacc, mybir

    f32 = mybir.dt.float32
    bf16 = mybir.dt.bfloat16
    f8 = mybir.dt.float8e4
    i32 = mybir.dt.int32
    AF = mybir.ActivationFunctionType
    OP = mybir.AluOpType
    DR = mybir.MatmulPerfMode.DoubleRow

    two_a = float(2.0 * a_f)
    rD = 1.0 / float(D)
    rD2 = rD * rD

    nc = bacc.Bacc("TRN2", target_bir_lowering=False, debug=False, enable_asserts=False)

    xh8_d = nc.dram_tensor("xh8", [D, N], f8, kind="ExternalInput").ap()
    xl8_d = nc.dram_tensor("xl8", [D, N], f8, kind="ExternalInput").ap()
    w8_d = nc.dram_tensor("w8", [D, 2 * D], f8, kind="ExternalInput").ap()
    xres_d = nc.dram_tensor("xres", [N, D], bf16, kind="ExternalInput").ap()
    rbias_d = nc.dram_tensor("rbias", [1, N], f32, kind="ExternalInput").ap()
    rprime_d = nc.dram_tensor("rprime", [1, N], f32, kind="ExternalInput").ap()
    brow_d = grow_d = berow_d = None
    if need_b:
        brow_d = nc.dram_tensor("brow", [D], f32, kind="ExternalInput").ap()
    if need_gamma:
        grow_d = nc.dram_tensor("grow", [D], f32, kind="ExternalInput").ap()
    if need_beta:
        berow_d = nc.dram_tensor("berow", [D], f32, kind="ExternalInput").ap()
    out_d = nc.dram_tensor("out", [N, D], bf16, kind="ExternalOutput").ap()

    def bcast_ap(src):
        return bass.AP(tensor=src.tensor, offset=src.offset,
                       ap=[[0, P]] + list(src.ap))

    with tile.TileContext(nc) as tc, ExitStack() as ctx:
        singles = ctx.enter_context(tc.tile_pool(name="singles", bufs=1))
        stats = ctx.enter_context(tc.tile_pool(name="stats", bufs=2))

        # ---- persistent SBUF ----
        xh8_sb = singles.tile([P, 2, N], f8)
        xl8_sb = singles.tile([P, 2, N], f8)
        w8_sb = singles.tile([P, 2, 2 * D], f8)
        xres_sb = singles.tile([P, NT, D], bf16)
        rbias_sb = singles.tile([P, NT], f32)
        rprime_sb = singles.tile([P, NT], f32)
        e8_sb = singles.tile([P, NT, N], f8)
        y8_sb = singles.tile([P, NT, D], f8)
        ybf_sb = singles.tile([P, NT, D], bf16)
        z_sb = singles.tile([P, NT, D], bf16)
        e_sb = singles.tile([P, NT, D], bf16)
        rx_sb = singles.tile([P, NT, D], bf16)
        v_sb = singles.tile([P, NT, D], bf16)
        vsqd_sb = singles.tile([P, 2, D], bf16)
        out_sb = singles.tile([P, NT, D], bf16)
        sv_sb = singles.tile([P, NT], f32)
        sv2_sb = singles.tile([P, NT], f32)
        qp_sb = singles.tile([P, NT], f32)
        ms_sb = singles.tile([P, NT], f32)
        wv_sb = singles.tile([P, NT], f32)
        sh_sb = singles.tile([P, NT], i32)
        r0_sb = singles.tile([P, NT], f32)
        a2_sb = singles.tile([P, NT], f32)
        b2_sb = singles.tile([P, NT], f32)
        rstd_sb = singles.tile([P, NT], f32)
        nmr_sb = singles.tile([P, NT], f32)
        magic_sb = singles.tile([P, NT], i32)

        warm_sb = singles.tile([P, 512], bf16)
        dummy_sb = singles.tile([P, 1], f32)

        nc.vector.memset(magic_sb[:], 0x5F3759DF)
        nc.vector.memset(dummy_sb[:], 0.0)

        # preload exp act table ASAP (overlaps the input DMA latency)
        nc.scalar.activation(dummy_sb[:], dummy_sb[:], AF.Exp)

        # ---- input DMAs first (urgency order), issued on 4 engines ----
        nc.sync.dma_start(out=xh8_sb[:],
                          in_=xh8_d.rearrange("(c p) n -> p c n", p=P))
        nc.gpsimd.dma_start(out=w8_sb[:],
                            in_=w8_d.rearrange("(c p) e -> p c e", p=P))
        nc.gpsimd.dma_start(out=xl8_sb[:],
                            in_=xl8_d.rearrange("(c p) n -> p c n", p=P))
        nc.scalar.dma_start(out=xres_sb[:],
                            in_=xres_d.rearrange("(t p) d -> p t d", p=P))
        nc.sync.dma_start(out=rbias_sb[:],
                          in_=rbias_d.rearrange("o (t p) -> p (o t)", p=P))
        nc.sync.dma_start(out=rprime_sb[:],
                          in_=rprime_d.rearrange("o (t p) -> p (o t)", p=P))
        b_bc = g_bc = be_bc = None
        if need_b:
            b_bc = singles.tile([P, D], f32)
            nc.scalar.dma_start(out=b_bc[:], in_=bcast_ap(brow_d))
        if need_gamma:
            g_bc = singles.tile([P, D], f32)
            nc.scalar.dma_start(out=g_bc[:], in_=bcast_ap(grow_d))
        if need_beta:
            be_bc = singles.tile([P, D], f32)
            nc.scalar.dma_start(out=be_bc[:], in_=bcast_ap(berow_d))

        nc.gpsimd.memset(warm_sb[:], 0.5)

        # ---- PE warmup while DMAs land ----
        warm_pool = tc.tile_pool(name="warm_psum", bufs=1, space="PSUM")
        wp = warm_pool.__enter__()
        pwarm = wp.tile([P, 512], f32)
        for _ in range(4):
            nc.tensor.matmul(pwarm[:], warm_sb[:, 0:P], warm_sb[:],
                             start=True, stop=True)
        warm_pool.__exit__(None, None, None)

        # ---- PSUM pools ----
        py_pool = tc.tile_pool(name="py_psum", bufs=1, space="PSUM")
        pyp = py_pool.__enter__()
        py_sb = pyp.tile([P, NT, D], f32)          # 4 banks, persistent
        g_pool = tc.tile_pool(name="g_psum", bufs=1, space="PSUM")
        gp = g_pool.__enter__()

        col0 = {0: 0, 1: HALF}

        def gram_half(h, pg_tiles):
            c0 = col0[h]
            for j in range(NT):
                pg = gp.tile([P, HALF], f32, tag=f"pg{j % 2}",
                             name=f"pg_{h}_{j}")
                pg_tiles[j] = pg
                nc.tensor.matmul(pg[:], xh8_sb[:, :, j * P:(j + 1) * P],
                                 xh8_sb[:, :, c0:c0 + HALF],
                                 start=True, stop=True, perf_mode=DR)

        def exp_half(h, pg_tiles):
            c0 = col0[h]
            for j in range(NT):
                nc.scalar.activation(
                    e8_sb[:, j, c0:c0 + HALF], pg_tiles[j][:], AF.Exp,
                    bias=rbias_sb[:, j:j + 1], scale=two_a,
                )
                dlo = j * P - c0      # patch diag block of E to 0 (gpsimd)
                if 0 <= dlo < HALF:
                    nc.gpsimd.affine_select(
                        out=e8_sb[:, j, j * P:(j + 1) * P],
                        in_=e8_sb[:, j, j * P:(j + 1) * P],
                        compare_op=OP.not_equal, fill=0,
                        base=0, channel_multiplier=1, pattern=[[-1, P]],
                    )

        # ---- pass A Gram ----
        pgA = {}
        gram_half(0, pgA)

        # ---- proj: y = x@W.T via fp8 hi/lo, into persistent py PSUM ----
        for j in range(NT):
            xb_h = xh8_sb[:, :, j * P:(j + 1) * P]
            xb_l = xl8_sb[:, :, j * P:(j + 1) * P]
            nc.tensor.matmul(py_sb[:, j, :], xb_h, w8_sb[:, :, 0:D],
                             start=True, stop=False, perf_mode=DR,
                             skip_group_check=True)
            nc.tensor.matmul(py_sb[:, j, :], xb_h, w8_sb[:, :, D:2 * D],
                             start=False, stop=False, perf_mode=DR,
                             skip_group_check=True)
            nc.tensor.matmul(py_sb[:, j, :], xb_l, w8_sb[:, :, 0:D],
                             start=False, stop=True, perf_mode=DR,
                             skip_group_check=True)

        exp_half(0, pgA)

        # y: PSUM -> bf16 on DVE (idle during Gram/exp of pass A), then
        # bf16 -> fp8 on gpsimd (gpsimd cannot read PSUM; DVE ops cannot
        # read two PSUM operands, so z consumes ybf not py)
        for j in range(NT):
            nc.vector.tensor_copy(ybf_sb[:, j, :], py_sb[:, j, :])
        for j in range(NT):
            nc.gpsimd.tensor_copy(y8_sb[:, j, :], ybf_sb[:, j, :])

        # ---- pass B Gram ----
        pgB = {}
        gram_half(1, pgB)

        # ---- aggregation + post chain ----
        a_pool = tc.tile_pool(name="a_psum", bufs=1, space="PSUM")
        ap_ = a_pool.__enter__()

        out_view = out_d.rearrange("(t p) d -> p t d", p=P)

        def newton(bs, eng=None):
            # rstd[bs] = rsqrt(sv2/D - (sv/D)^2 + eps), 1 Newton iter (DVE)
            eng = nc.vector
            eng.tensor_scalar(qp_sb[:, bs], sv2_sb[:, bs], rD, LN_EPS,
                              OP.mult, OP.add)
            eng.tensor_tensor(ms_sb[:, bs], sv_sb[:, bs], sv_sb[:, bs], OP.mult)
            eng.scalar_tensor_tensor(wv_sb[:, bs], ms_sb[:, bs], -rD2,
                                     qp_sb[:, bs], OP.mult, OP.add)
            eng.tensor_scalar(sh_sb[:, bs], wv_sb[:, bs].bitcast(i32), 1, None,
                              OP.arith_shift_right)
            eng.tensor_tensor(r0_sb[:, bs].bitcast(i32), magic_sb[:, bs],
                              sh_sb[:, bs], OP.subtract)
            eng.tensor_tensor(a2_sb[:, bs], r0_sb[:, bs], r0_sb[:, bs], OP.mult)
            eng.scalar_tensor_tensor(b2_sb[:, bs], a2_sb[:, bs], -0.5,
                                     wv_sb[:, bs], OP.mult, OP.mult)
            eng.scalar_tensor_tensor(rstd_sb[:, bs], b2_sb[:, bs], 1.5,
                                     r0_sb[:, bs], OP.add, OP.mult)
            eng.scalar_tensor_tensor(nmr_sb[:, bs], sv_sb[:, bs], -rD,
                                     rstd_sb[:, bs], OP.mult, OP.mult)

        def agg_half(h):
            i0 = 4 * h
            pas = {}
            for di in range(4):
                i = i0 + di
                pas[i] = ap_.tile([P, D], f32, tag=f"pa{di % 2}",
                                  name=f"pa_{h}_{di}")
                for jp in range(4):
                    nc.tensor.matmul(
                        pas[i][:],
                        e8_sb[:, 2 * jp:2 * jp + 2, i * P:(i + 1) * P],
                        y8_sb[:, 2 * jp:2 * jp + 2, :],
                        start=(jp == 0),
                        stop=(jp == 3),
                        perf_mode=DR, skip_group_check=True,
                    )
            return pas

        def post_z(i, pa):
            # z = R'*pa + y  (one STT: PSUM agg read + SBUF bf16 y)
            nc.vector.scalar_tensor_tensor(
                z_sb[:, i, :], pa[:], rprime_sb[:, i:i + 1], ybf_sb[:, i, :],
                OP.mult, OP.add)
            if need_b:
                nc.vector.tensor_tensor(z_sb[:, i, :], z_sb[:, i, :], b_bc[:],
                                        OP.add)

        def post_elu_pair(i):
            nc.scalar.activation(e_sb[:, i:i + 2, :], z_sb[:, i:i + 2, :],
                                 AF.Exp)

        def post_rest(i):
            nc.vector.scalar_tensor_tensor(
                rx_sb[:, i, :], z_sb[:, i, :], 0.0, xres_sb[:, i, :],
                OP.max, OP.add)
            nc.vector.scalar_tensor_tensor(
                v_sb[:, i, :], e_sb[:, i, :], 1.0, rx_sb[:, i, :],
                OP.min, OP.add, accum_out=sv_sb[:, i:i + 1])
            nc.vector.scalar_tensor_tensor(
                vsqd_sb[:, i % 2, :], v_sb[:, i, :], 1.0, v_sb[:, i, :],
                OP.mult, OP.mult, accum_out=sv2_sb[:, i:i + 1])

        def post_norm(i, eng):
            eng.tensor_scalar(
                out_sb[:, i, :], v_sb[:, i, :], rstd_sb[:, i:i + 1],
                nmr_sb[:, i:i + 1], OP.mult, OP.add)
            if need_gamma:
                eng.tensor_mul(out_sb[:, i, :], out_sb[:, i, :], g_bc[:])
            if need_beta:
                eng.tensor_add(out_sb[:, i, :], out_sb[:, i, :], be_bc[:])
            if eng is nc.vector:
                nc.sync.dma_start(out=out_view[:, i, :], in_=out_sb[:, i, :])
            else:
                nc.gpsimd.dma_start(out=out_view[:, i, :], in_=out_sb[:, i, :])

        # pass A agg + z (emitted before exp-B so the pass-A elu pairs can
        # statically interleave into the ACT exp-B stream)
        pasA = agg_half(0)
        for i in range(4):
            post_z(i, pasA[i])

        # exp-B with pass-A elu pairs woven into the ACT stream
        c0 = HALF
        for j in range(NT):
            nc.scalar.activation(
                e8_sb[:, j, c0:c0 + HALF], pgB[j][:], AF.Exp,
                bias=rbias_sb[:, j:j + 1], scale=two_a,
            )
            if j >= 4:
                nc.gpsimd.affine_select(
                    out=e8_sb[:, j, j * P:(j + 1) * P],
                    in_=e8_sb[:, j, j * P:(j + 1) * P],
                    compare_op=OP.not_equal, fill=0,
                    base=0, channel_multiplier=1, pattern=[[-1, P]],
                )
            if j == 1:
                post_elu_pair(0)
            elif j == 3:
                post_elu_pair(2)

        for i in range(4):
            post_rest(i)
        newton(slice(0, 4))
        # norms 0-3 fill DVE idle time while pass-B agg runs
        for i in range(4):
            post_norm(i, nc.vector)

        # pass B
        pasB = agg_half(1)
        for i in range(4, 8):
            post_z(i, pasB[i])
        post_elu_pair(4)
        post_elu_pair(6)
        for i in range(4, 8):
            post_rest(i)
        newton(slice(4, 8))
        for i in range(4, 8):
            post_norm(i, nc.vector)

        a_pool.__exit__(None, None, None)
        g_pool.__exit__(None, None, None)
        py_pool.__exit__(None, None, None)

    nc.compile()
    return nc


def _prepare_core_inputs_fast(x_k, a_f, w_f, W_T, b_proj, ln_gamma, ln_beta,
                              need_b, need_gamma, need_beta):
    import ml_dtypes
    from concourse import mybir

    bf = ml_dtypes.bfloat16
    f8 = mybir.dt.np(mybir.dt.float8e4)
    xf = np.ascontiguousarray(x_k, dtype=np.float32)
    sq = np.sum(xf.astype(np.float64) ** 2, axis=-1)
    a64 = np.float64(a_f)
    K = float(a64 * np.mean(sq))
    xT = np.ascontiguousarray(xf.T)
    xh8 = xT.astype(f8)
    xl8 = (xT - xh8.astype(np.float32)).astype(f8)
    wh8 = W_T.astype(f8)
    wl8 = (W_T - wh8.astype(np.float32)).astype(f8)
    m = {
        "xh8": xh8,
        "xl8": xl8,
        "w8": np.concatenate([wh8, wl8], axis=1),
        "xres": (xf - np.float32(1.0)).astype(bf),
        "rbias": (-(a64 * sq + K)).astype(np.float32).reshape(1, N),
        "rprime": (np.float64(w_f) * np.exp(K - a64 * sq)).astype(
            np.float32).reshape(1, N),
    }
    if need_b:
        m["brow"] = b_proj
    if need_gamma:
        m["grow"] = ln_gamma
    if need_beta:
        m["berow"] = ln_beta
    return m


def _specialize(inputs):
    x = np.asarray(inputs["x"], dtype=np.float32)
    log_sigmas = np.asarray(inputs["log_sigmas"], dtype=np.float32)
    W_proj = np.asarray(inputs["W_proj"], dtype=np.float32)
    b_proj = np.ascontiguousarray(np.asarray(inputs["b_proj"], dtype=np.float32))
    ln_gamma = np.ascontiguousarray(np.asarray(inputs["ln_gamma"], dtype=np.float32))
    ln_beta = np.ascontiguousarray(np.asarray(inputs["ln_beta"], dtype=np.float32))

    sigmas = np.exp(log_sigmas)
    denoms = (np.float32(2.0) * sigmas * sigmas + np.float32(1e-6)).astype(np.float32)
    uniq, counts = np.unique(denoms, return_counts=True)
    alphas = (np.float32(1.0) / uniq).astype(np.float32)
    weights = counts.astype(np.float32) / np.float32(H)

    need_b = bool(np.any(b_proj != 0))
    need_gamma = not bool(np.all(ln_gamma == 1))
    need_beta = bool(np.any(ln_beta != 0))
    return (x, W_proj, b_proj, ln_gamma, ln_beta, alphas, weights,
            need_b, need_gamma, need_beta)


def kernel(**inputs):
    from concourse import bass_utils

    (x, W_proj, b_proj, ln_gamma, ln_beta, alphas, weights,
     need_b, need_gamma, need_beta) = _specialize(inputs)
    assert x.shape == (B, N, D), x.shape
    assert len(alphas) == 1, "fast path requires single unique sigma"
    a_f = float(alphas[0])
    w_f = float(weights[0])

    key = (a_f, w_f, need_b, need_gamma, need_beta)
    if key not in _PROGRAM_CACHE:
        _PROGRAM_CACHE[key] = _build_fast(a_f, w_f, 0.0, need_b, need_gamma,
                                          need_beta)
    nc = _PROGRAM_CACHE[key]

    W_T = np.ascontiguousarray(W_proj.T)
    in_maps = [
        _prepare_core_inputs_fast(x[k], a_f, w_f, W_T, b_proj, ln_gamma,
                                  ln_beta, need_b, need_gamma, need_beta)
        for k in range(B)
    ]
    res = bass_utils.run_bass_kernel_spmd(nc, in_maps, core_ids=list(range(B)))
    out = np.stack([res.results[k]["out"] for k in range(B)])
    return out.astype(np.float32)


if __name__ == "__main__":
    import reference as R

    inp = R.setup_inputs()
    got = kernel(**{k: np.asarray(v) for k, v in inp.items()})
    print("out shape", got.shape, got.dtype)
